# revision 1
# baseline (speedup 1.0000x reference)
"""Trainium2 Bass kernel for LANLayer-style GNN message passing.

Reference computation (N=8192, DIM=256, HID=128, K=10):
    h = relu(x @ W_proj + b); hn = h / ||h||
    sim = (hn @ hn.T + 1)/2; probs = softmax(sim/T); topi = top_k(probs, 10)
    A_hat = one_hot(topi) minus diag plus eye; deg = 10 per row
    out = relu((A_hat/10) @ (h @ W_conv))

Key algebraic reductions used here:
  - softmax is rank-preserving per row -> top-k of probs == top-k of
    Z[i,j] = h_i . hn_j  (query unnormalized, key normalized).
  - The diagonal is always top-1 (Z[i,i] = ||h_i||), and the reference's
    "scatter, zero diag, add eye" keeps exactly the top-10 set, so every
    row degree is 10 and adj_norm = A_hat / 10.
  - Row-wise 10th/11th largest found exactly via per-512-group max8 +
    mini top-16 on the group winners; mask = (Z >= (v10+v11)/2).
  - out rows = relu(0.1 * mask @ support), computed as a dense bf16
    matmul with PE-transposed mask blocks (mask is exact 0/1 in bf16).

Sharding: 8 cores; all inputs replicated except each core also gets its
own 1024-row slice of x ("xloc") so one SPMD program computes rows
[c*1024, (c+1)*1024) without dynamic addressing.
"""

import numpy as np

import concourse.bass as bass
import concourse.mybir as mybir
import concourse.tile as tile
from concourse import bacc
from concourse.bass_utils import run_bass_kernel_spmd
from concourse.masks import make_identity

N, DIM, HID = 8192, 256, 128
NCORES = 8
NLOC = N // NCORES          # 1024 rows per core
CH = 512                    # node chunk for phase 0 / Z matmul rhs
NCH = N // CH               # 16
RT = NLOC // 128            # 8 row-tiles per core
JT = N // 128               # 64 j-tiles
GROUP = 2                   # row-tiles aggregated together in phase 2
F32 = mybir.dt.float32
BF16 = mybir.dt.bfloat16
NEG = -1.0e30


def _transpose_x_chunk(nc, pool, psum_tp, ident, dram_x, base_row, xT_sb):
    """Load 512 rows of x and produce xT chunk [256(2x128 part), 512] in SBUF."""
    xrows = []
    for r in range(4):
        t = pool.tile([128, DIM], F32, tag="xrow")
        nc.sync.dma_start(t[:], dram_x[base_row + r * 128 : base_row + (r + 1) * 128, :])
        xrows.append(t)
    for fb in range(2):
        pt = psum_tp.tile([128, 512], F32, tag="xtp")
        for r in range(4):
            nc.tensor.transpose(
                pt[:, r * 128 : (r + 1) * 128],
                xrows[r][:, fb * 128 : (fb + 1) * 128],
                ident[:],
            )
        nc.scalar.copy(xT_sb[fb][:], pt[:])


def build_nc():
    nc = bacc.Bacc(None, target_bir_lowering=False)

    x_t = nc.dram_tensor("x", [N, DIM], F32, kind="ExternalInput")
    xloc_t = nc.dram_tensor("xloc", [NLOC, DIM], F32, kind="ExternalInput")
    wp_t = nc.dram_tensor("W_proj", [DIM, HID], F32, kind="ExternalInput")
    bp_t = nc.dram_tensor("b_proj", [HID], F32, kind="ExternalInput")
    wc_t = nc.dram_tensor("W_conv", [HID, HID], F32, kind="ExternalInput")
    out_t = nc.dram_tensor("out", [NLOC, HID], F32, kind="ExternalOutput")

    with tile.TileContext(nc) as tc:
        with (
            tc.tile_pool(name="const", bufs=1) as cpool,
            tc.tile_pool(name="big", bufs=1) as big,
        ):
            # --- constants ---
            ident = cpool.tile([128, 128], F32)
            make_identity(nc, ident[:])
            identb = cpool.tile([128, 128], BF16)
            make_identity(nc, identb[:])
            ones_col = cpool.tile([128, 1], F32)
            nc.vector.memset(ones_col[:], 1.0)
            ones_row = cpool.tile([1, 128], F32)
            nc.vector.memset(ones_row[:], 1.0)
            wp_sb = cpool.tile([128, 2, HID], F32)
            nc.sync.dma_start(wp_sb[:], wp_t[:].rearrange("(k p) h -> p k h", p=128))
            b_sb = cpool.tile([128, 1], F32)
            nc.sync.dma_start(b_sb[:], bp_t[:].rearrange("(p one) -> p one", one=1))
            wc_sb = cpool.tile([128, HID], F32)
            nc.sync.dma_start(wc_sb[:], wc_t[:])

            # --- persistent big tensors ---
            hnT = big.tile([128, N], F32)          # normalized keys, [hid, j]
            hTloc = big.tile([128, NLOC], F32)     # unnormalized queries, [hid, i]
            supp = big.tile([128, JT, HID], BF16)  # support rows, [j%128, jt, hid]

            # ---------------- phase 0 ----------------
            with (
                tc.tile_pool(name="ph0", bufs=5) as p0,
                tc.tile_pool(name="ph0b", bufs=3) as p0b,
                tc.tile_pool(name="hTpool", bufs=1) as hpool,
                tc.tile_pool(name="ph0psum", bufs=3, space="PSUM") as pp_tp,
                tc.tile_pool(name="ph0psum2", bufs=3, space="PSUM") as pp_mm,
                tc.tile_pool(name="ph0psum3", bufs=2, space="PSUM") as pp_n2,
            ):
                hT = hpool.tile([128, N], F32)     # unnormalized, [hid, j]

                for c in range(NCH):
                    xT_sb = [p0.tile([128, 512], F32, tag=f"xT{fb}", name=f"xT{fb}") for fb in range(2)]
                    _transpose_x_chunk(nc, p0, pp_tp, ident, x_t, c * CH, xT_sb)
                    hp = pp_mm.tile([128, 512], F32, tag="hmm")
                    nc.tensor.matmul(hp[:], wp_sb[:, 0, :], xT_sb[0][:], start=True, stop=False)
                    nc.tensor.matmul(hp[:], wp_sb[:, 1, :], xT_sb[1][:], start=False, stop=True)
                    sl = slice(c * CH, (c + 1) * CH)
                    nc.scalar.activation(
                        hT[:, sl], hp[:], mybir.ActivationFunctionType.Relu, bias=b_sb[:]
                    )
                    # row norms^2 via ones-matmul over the hid (partition) dim
                    sq = p0b.tile([128, 512], F32, tag="sq")
                    nc.vector.tensor_mul(sq[:], hT[:, sl], hT[:, sl])
                    n2 = pp_n2.tile([1, 512], F32, tag="n2")
                    nc.tensor.matmul(n2[:], ones_col[:], sq[:], start=True, stop=True)
                    rchunk = p0b.tile([1, 512], F32, tag="rchunk")
                    nc.scalar.activation(
                        rchunk[:], n2[:], mybir.ActivationFunctionType.Sqrt
                    )
                    nc.vector.tensor_scalar_max(rchunk[:], rchunk[:], 1e-12)
                    nc.vector.reciprocal(rchunk[:], rchunk[:])
                    rb = pp_mm.tile([128, 512], F32, tag="hmm", name="rb")
                    nc.tensor.matmul(rb[:], ones_row[:], rchunk[:], start=True, stop=True)
                    nc.vector.tensor_mul(hnT[:, sl], hT[:, sl], rb[:])

                # support = h @ W_conv, row-major bf16: supp[:, jt, :]
                for jt in range(JT):
                    sp = pp_tp.tile([128, 512], F32, tag="xtp", name="sp")
                    nc.tensor.matmul(
                        sp[:, :HID], hT[:, jt * 128 : (jt + 1) * 128], wc_sb[:], start=True, stop=True
                    )
                    nc.scalar.copy(supp[:, jt, :], sp[:, :HID])

                # local (query) columns: recompute h for own rows from xloc
                for c in range(2):
                    xT_sb = [p0.tile([128, 512], F32, tag=f"xT{fb}", name=f"xT{fb}") for fb in range(2)]
                    _transpose_x_chunk(nc, p0, pp_tp, ident, xloc_t, c * CH, xT_sb)
                    hp = pp_mm.tile([128, 512], F32, tag="hmm")
                    nc.tensor.matmul(hp[:], wp_sb[:, 0, :], xT_sb[0][:], start=True, stop=False)
                    nc.tensor.matmul(hp[:], wp_sb[:, 1, :], xT_sb[1][:], start=False, stop=True)
                    nc.scalar.activation(
                        hTloc[:, c * CH : (c + 1) * CH],
                        hp[:],
                        mybir.ActivationFunctionType.Relu,
                        bias=b_sb[:],
                    )

            # ---------------- phases 1+2, grouped ----------------
            with (
                tc.tile_pool(name="zpsum", bufs=3, space="PSUM") as zp,
                tc.tile_pool(name="zsb", bufs=2) as zpool,
                tc.tile_pool(name="small", bufs=4) as sm,
                tc.tile_pool(name="masks", bufs=2 * GROUP) as mpool,
                tc.tile_pool(name="atpsum", bufs=3, space="PSUM") as atp,
                tc.tile_pool(name="atsb", bufs=6) as atsb,
                tc.tile_pool(name="opsum", bufs=1, space="PSUM") as op,
                tc.tile_pool(name="outsb", bufs=2) as osb,
            ):
                for g in range(RT // GROUP):
                    gmasks = []
                    for rt_in in range(GROUP):
                        rt = g * GROUP + rt_in
                        isl = slice(rt * 128, (rt + 1) * 128)
                        # Z row-tile: [128 i, 8192 j]
                        z_sb = zpool.tile([128, N], F32, tag="z")
                        m8 = sm.tile([128, NCH, 8], F32, tag="m8")
                        for c in range(NCH):
                            zps = zp.tile([128, 512], F32, tag="zp")
                            nc.tensor.matmul(
                                zps[:], hTloc[:, isl], hnT[:, c * CH : (c + 1) * CH],
                                start=True, stop=True,
                            )
                            sl = slice(c * CH, (c + 1) * CH)
                            nc.scalar.copy(z_sb[:, sl], zps[:])
                            nc.vector.max(m8[:, c, :], z_sb[:, sl])
                        # mini top-16 over the 128 group winners
                        t8a = sm.tile([128, 8], F32, tag="t8a")
                        m8z = sm.tile([128, NCH, 8], F32, tag="m8z")
                        t8b = sm.tile([128, 8], F32, tag="t8b")
                        tau = sm.tile([128, 1], F32, tag="tau")
                        m8f = m8[:].rearrange("p a b -> p (a b)")
                        m8zf = m8z[:].rearrange("p a b -> p (a b)")
                        nc.vector.max(t8a[:], m8f)
                        nc.vector.match_replace(m8zf, t8a[:], m8f, NEG)
                        nc.vector.max(t8b[:], m8zf)
                        # tau = (v10 + v11)/2 ; v10 = t8b[:,1], v11 = t8b[:,2]
                        nc.vector.tensor_add(tau[:], t8b[:, 1:2], t8b[:, 2:3])
                        nc.vector.tensor_scalar_mul(tau[:], tau[:], 0.5)
                        mask = mpool.tile([128, N], BF16, tag="mask")
                        nc.vector.tensor_scalar(
                            mask[:], z_sb[:], tau[:], None, op0=mybir.AluOpType.is_ge
                        )
                        gmasks.append(mask)

                    # phase 2: out rows for this group of 4 row-tiles
                    oTa = op.tile([128, GROUP * 128], F32, tag="oTa")
                    oTb = op.tile([128, GROUP * 128], F32, tag="oTb")
                    for jt in range(JT):
                        at_ps = atp.tile([128, GROUP * 128], BF16, tag="at")
                        for rt_in in range(GROUP):
                            nc.tensor.transpose(
                                at_ps[:, rt_in * 128 : (rt_in + 1) * 128],
                                gmasks[rt_in][:, jt * 128 : (jt + 1) * 128],
                                identb[:],
                            )
                        at_s = atsb.tile([128, GROUP * 128], BF16, tag="ats")
                        if jt % 2 == 0:
                            nc.vector.tensor_copy(at_s[:], at_ps[:])
                        else:
                            nc.scalar.copy(at_s[:], at_ps[:])
                        acc = oTa if jt % 2 == 0 else oTb
                        nc.tensor.matmul(
                            acc[:], supp[:, jt, :], at_s[:],
                            start=(jt < 2), stop=(jt >= JT - 2),
                        )
                    # relu(0.1 * (oTa + oTb)), transpose back to [i, hid], DMA out
                    ob_sb = osb.tile([128, GROUP * 128], F32, tag="obsb")
                    nc.scalar.copy(ob_sb[:], oTb[:])
                    osum = osb.tile([128, GROUP * 128], F32, tag="osum")
                    nc.vector.tensor_add(osum[:], oTa[:], ob_sb[:])
                    oT_sb = osb.tile([128, GROUP * 128], F32, tag="oTsb")
                    nc.scalar.activation(
                        oT_sb[:], osum[:], mybir.ActivationFunctionType.Relu, scale=0.1
                    )
                    for rt_in in range(GROUP):
                        ops_ = atp.tile([128, GROUP * 128], F32, tag="at", name="ops_")
                        nc.tensor.transpose(
                            ops_[:, :128], oT_sb[:, rt_in * 128 : (rt_in + 1) * 128], ident[:]
                        )
                        o_sb = osb.tile([128, 128], F32, tag="osb")
                        nc.scalar.copy(o_sb[:], ops_[:, :128])
                        r0 = (g * GROUP + rt_in) * 128
                        nc.sync.dma_start(out_t[r0 : r0 + 128, :], o_sb[:])

    nc.compile()
    return nc


_NC_CACHE = {}


def kernel(x, W_proj, b_proj, W_conv):
    if "nc" not in _NC_CACHE:
        _NC_CACHE["nc"] = build_nc()
    nc = _NC_CACHE["nc"]
    x = np.ascontiguousarray(x, dtype=np.float32)
    in_maps = []
    for c in range(NCORES):
        in_maps.append(
            {
                "x": x,
                "xloc": np.ascontiguousarray(x[c * NLOC : (c + 1) * NLOC]),
                "W_proj": np.ascontiguousarray(W_proj, dtype=np.float32),
                "b_proj": np.ascontiguousarray(b_proj, dtype=np.float32),
                "W_conv": np.ascontiguousarray(W_conv, dtype=np.float32),
            }
        )
    res = run_bass_kernel_spmd(nc, in_maps, core_ids=list(range(NCORES)))
    return np.concatenate([res.results[c]["out"] for c in range(NCORES)], axis=0)

